# revision 5
# baseline (speedup 1.0000x reference)
"""KAN forward kernel for Trainium2 (8 NeuronCores, data-parallel over N).

Math (per sample n):
  h[o,q,p,hh]  = tanh(x[p] * W1[o,q,p,hh] + b1[o,q,p,hh])
  pre[o,q]     = sum_{p,hh} h * W2[o,q,p,hh]            (+ sum_p b2[o,q,p])
  ho[o,q,hh]   = tanh(pre * V1[o,q,hh] + c1[o,q,hh])
  out[o]       = sum_{q,hh} ho * V2[o,q,hh]             (+ sum_q c2[o,q])

Device mapping (per core, Nc = 4096 samples):
  - partitions = (p,hh) = 8*16 = 128 exactly; free dim = n.
  - layer-1 mul+add fuses into the ScalarE activation (per-partition
    scale/bias), one tanh instruction per (o,q) pair -> 68 instrs.
  - the (p,hh) reduction runs on TensorE: per (o,q) a masked stationary
    matrix (only column oq nonzero) so 68 accumulating matmuls build
    pre[(o,q), n] in PSUM without any transposes.
  - layer 2: 16 tanh instructions (partition = (o,q) = 68), then masked
    matmuls reduce (q,hh) -> out[o, n].
"""

from contextlib import ExitStack

import ml_dtypes
import numpy as np

O, Q, P, H = 4, 17, 8, 16
OQ = O * Q  # 68
PH = P * H  # 128
N_CORES = 8
N = 32768
NC = N // N_CORES  # 4096
MM_N = 512  # moving free dim per matmul == one PSUM bank of fp32

_CACHE = {}


def _build():
    import concourse.bass as bass
    import concourse.tile as tile
    from concourse import bacc, mybir

    F32 = mybir.dt.float32
    BF16 = mybir.dt.bfloat16
    Tanh = mybir.ActivationFunctionType.Tanh

    nc = bacc.Bacc("TRN2", target_bir_lowering=False, debug=False)

    xd = nc.dram_tensor("x_rep", [PH, NC], F32, kind="ExternalInput")
    w1d = nc.dram_tensor("w1col", [PH, OQ], F32, kind="ExternalInput")
    b1d = nc.dram_tensor("b1col", [PH, OQ], F32, kind="ExternalInput")
    w2d = nc.dram_tensor("w2mask", [PH, OQ * OQ], BF16, kind="ExternalInput")
    v1d = nc.dram_tensor("v1col", [OQ, H], F32, kind="ExternalInput")
    bi2d = nc.dram_tensor("bias2", [OQ, H], F32, kind="ExternalInput")
    v2d = nc.dram_tensor("v2mask", [OQ, H * O], BF16, kind="ExternalInput")
    c2d = nc.dram_tensor("c2sum", [O, 1], F32, kind="ExternalInput")
    outd = nc.dram_tensor("out", [O, NC], F32, kind="ExternalOutput")

    with tile.TileContext(nc) as tc, ExitStack() as ctx:
        const = ctx.enter_context(tc.tile_pool(name="const", bufs=1))
        hpool = ctx.enter_context(tc.tile_pool(name="h", bufs=4))
        hopool = ctx.enter_context(tc.tile_pool(name="ho", bufs=3))
        sbpool = ctx.enter_context(tc.tile_pool(name="sb", bufs=1))

        xr = const.tile([PH, NC], F32)
        nc.sync.dma_start(out=xr[:], in_=xd[:])
        w1c = const.tile([PH, OQ], F32)
        nc.sync.dma_start(out=w1c[:], in_=w1d[:])
        b1c = const.tile([PH, OQ], F32)
        nc.sync.dma_start(out=b1c[:], in_=b1d[:])
        w2m = const.tile([PH, OQ * OQ], BF16)
        nc.sync.dma_start(out=w2m[:], in_=w2d[:])
        v1c = const.tile([OQ, H], F32)
        nc.sync.dma_start(out=v1c[:], in_=v1d[:])
        bi2 = const.tile([OQ, H], F32)
        nc.sync.dma_start(out=bi2[:], in_=bi2d[:])
        v2m = const.tile([OQ, H * O], BF16)
        nc.sync.dma_start(out=v2m[:], in_=v2d[:])
        c2s = const.tile([O, 1], F32)
        nc.sync.dma_start(out=c2s[:], in_=c2d[:])

        # ---- layer 1: h = tanh(W1*x + b1), reduce (p,hh) with W2 -> pre ----
        pre_sb = sbpool.tile([OQ, NC], F32)
        with tc.tile_pool(name="pre", bufs=1, space="PSUM") as prepool:
            pre = prepool.tile([OQ, NC], F32)
            for oq in range(OQ):
                h = hpool.tile([PH, NC], BF16, tag="h")
                nc.scalar.activation(
                    out=h[:],
                    in_=xr[:],
                    func=Tanh,
                    bias=b1c[:, oq : oq + 1],
                    scale=w1c[:, oq : oq + 1],
                )
                for j in range(NC // MM_N):
                    nc.tensor.matmul(
                        pre[:, j * MM_N : (j + 1) * MM_N],
                        w2m[:, oq * OQ : (oq + 1) * OQ],
                        h[:, j * MM_N : (j + 1) * MM_N],
                        start=(oq == 0),
                        stop=(oq == OQ - 1),
                    )

            # free all 8 PSUM banks for the layer-2 output accumulators
            nc.vector.tensor_copy(out=pre_sb[:], in_=pre[:])

        # ---- layer 2: ho = tanh(V1*pre + bias2), reduce (q,hh) with V2 ----
        outsb = sbpool.tile([O, NC], F32)
        with tc.tile_pool(name="ops", bufs=8, space="PSUM") as opspool:
            ops_tiles = [
                opspool.tile([O, MM_N], F32, tag="ops", name=f"ops_{j}")
                for j in range(NC // MM_N)
            ]
            for hh in range(H):
                ho = hopool.tile([OQ, NC], BF16, tag="ho")
                nc.scalar.activation(
                    out=ho[:],
                    in_=pre_sb[:],
                    func=Tanh,
                    bias=bi2[:, hh : hh + 1],
                    scale=v1c[:, hh : hh + 1],
                )
                for j in range(NC // MM_N):
                    nc.tensor.matmul(
                        ops_tiles[j][:],
                        v2m[:, hh * O : (hh + 1) * O],
                        ho[:, j * MM_N : (j + 1) * MM_N],
                        start=(hh == 0),
                        stop=(hh == H - 1),
                    )
            for j in range(NC // MM_N):
                nc.vector.tensor_scalar_add(
                    out=outsb[:, j * MM_N : (j + 1) * MM_N],
                    in0=ops_tiles[j][:],
                    scalar1=c2s[:],
                )
        nc.sync.dma_start(out=outd[:], in_=outsb[:])

    nc.compile()
    return nc


def _prep_inputs(x, W1, b1, W2, b2, V1, c1, V2, c2):
    bf16 = ml_dtypes.bfloat16
    f32 = np.float32
    x = np.asarray(x, f32)
    # x_rep[c][(p*H+hh), n] = x[c*NC+n, p]
    xr = x.reshape(N_CORES, NC, P).transpose(0, 2, 1)  # (cores, P, NC)
    x_rep = np.ascontiguousarray(np.repeat(xr, H, axis=1), dtype=f32)

    w1col = np.ascontiguousarray(
        np.asarray(W1, f32).transpose(2, 3, 0, 1).reshape(PH, OQ)
    )
    b1col = np.ascontiguousarray(
        np.asarray(b1, f32).transpose(2, 3, 0, 1).reshape(PH, OQ)
    )
    w2t = np.asarray(W2, f32).transpose(2, 3, 0, 1).reshape(PH, OQ)
    idx = np.arange(OQ)
    w2mask = np.zeros((PH, OQ, OQ), f32)
    w2mask[:, idx, idx] = w2t
    w2mask = np.ascontiguousarray(w2mask.reshape(PH, OQ * OQ)).astype(bf16)

    b2sum = np.asarray(b2, f32).sum(axis=2).reshape(OQ)
    v1col = np.ascontiguousarray(np.asarray(V1, f32).reshape(OQ, H))
    bias2 = np.ascontiguousarray(
        np.asarray(c1, f32).reshape(OQ, H) + v1col * b2sum[:, None]
    )
    v2r = np.asarray(V2, f32).reshape(OQ, H)
    o_of = np.repeat(np.arange(O), Q)
    v2mask = np.zeros((OQ, H, O), f32)
    v2mask[idx, :, o_of] = v2r
    v2mask = np.ascontiguousarray(v2mask.reshape(OQ, H * O)).astype(bf16)

    c2sum = np.ascontiguousarray(np.asarray(c2, f32).sum(axis=1).reshape(O, 1))

    shared = {
        "w1col": w1col,
        "b1col": b1col,
        "w2mask": w2mask,
        "v1col": v1col,
        "bias2": bias2,
        "v2mask": v2mask,
        "c2sum": c2sum,
    }
    in_maps = [dict(shared, x_rep=np.ascontiguousarray(x_rep[c])) for c in range(N_CORES)]
    return in_maps


def run_spmd(x, W1, b1, W2, b2, V1, c1, V2, c2, trace=False):
    """Compile (cached), run on 8 cores, return (out_full, BassKernelResults)."""
    from concourse.bass_utils import run_bass_kernel_spmd

    if "nc" not in _CACHE:
        _CACHE["nc"] = _build()
    nc = _CACHE["nc"]
    in_maps = _prep_inputs(x, W1, b1, W2, b2, V1, c1, V2, c2)
    res = run_bass_kernel_spmd(nc, in_maps, list(range(N_CORES)), trace=trace)
    out_full = np.empty((N, O), dtype=np.float32)
    for c in range(N_CORES):
        out_full[c * NC : (c + 1) * NC, :] = res.results[c]["out"].T
    return out_full, res


def kernel(x, W1, b1, W2, b2, V1, c1, V2, c2):
    out, _ = run_spmd(x, W1, b1, W2, b2, V1, c1, V2, c2, trace=False)
    return out


# revision 8
# speedup vs baseline: 1.0834x; 1.0834x over previous
"""KAN forward kernel for Trainium2 (8 NeuronCores, data-parallel over N).

Math (per sample n):
  h[o,q,p,hh]  = tanh(x[p] * W1[o,q,p,hh] + b1[o,q,p,hh])
  pre[o,q]     = sum_{p,hh} h * W2[o,q,p,hh]            (+ sum_p b2[o,q,p])
  ho[o,q,hh]   = tanh(pre * V1[o,q,hh] + c1[o,q,hh])
  out[o]       = sum_{q,hh} ho * V2[o,q,hh]             (+ sum_q c2[o,q])

Device mapping (per core, Nc = 4096 samples):
  - partitions = (p,hh) = 8*16 = 128 exactly; free dim = n.
  - layer-1 mul+add fuses into the ScalarE activation (per-partition
    scale/bias), one tanh instruction per (o,q) pair -> 68 instrs.
  - the (p,hh) reduction runs on TensorE with masked stationary weights,
    M=128 wide so PSUM rows 68..127 come out pre-duplicated with
    pre[oq] for oq<60 (free: same banks, same stream time).  That gives
    the layer-2 tanh full 128-partition packing: 8 "pair" instructions
    cover (oq 0..67, even hh) + (oq 0..59, odd hh); one leftover
    instruction covers (oq 60..67, odd hh) from a DVE-replicated tile.
  - layer-2 reduction: masked matmuls (K=128 / K=64) -> out[o, n].
"""

from contextlib import ExitStack

import ml_dtypes
import numpy as np

O, Q, P, H = 4, 17, 8, 16
OQ = O * Q  # 68
PH = P * H  # 128
N_CORES = 8
N = 32768
NC = N // N_CORES  # 4096
MM_N = 512  # moving free dim per matmul == one PSUM bank of fp32
NJ = NC // MM_N  # 8 column chunks
NPAIR = H // 2  # 8 pair instructions
DUP = PH - OQ  # 60 duplicated oq rows
TAIL_OQ = OQ - DUP  # 8 leftover oq (60..67)
TAIL_P = TAIL_OQ * NPAIR  # 64 partitions in the leftover instr

_CACHE = {}


def _build():
    import concourse.bass as bass
    import concourse.tile as tile
    from concourse import bacc, mybir

    F32 = mybir.dt.float32
    BF16 = mybir.dt.bfloat16
    Tanh = mybir.ActivationFunctionType.Tanh

    nc = bacc.Bacc("TRN2", target_bir_lowering=False, debug=False)

    xd = nc.dram_tensor("x_rep", [PH, NC], F32, kind="ExternalInput")
    w1d = nc.dram_tensor("w1col", [PH, OQ], F32, kind="ExternalInput")
    b1d = nc.dram_tensor("b1col", [PH, OQ], F32, kind="ExternalInput")
    w2d = nc.dram_tensor("w2mask", [PH, OQ * PH], BF16, kind="ExternalInput")
    psd = nc.dram_tensor("pair_scale", [PH, NPAIR], F32, kind="ExternalInput")
    pbd = nc.dram_tensor("pair_bias", [PH, NPAIR], F32, kind="ExternalInput")
    tsd = nc.dram_tensor("tail_scale", [TAIL_P, 1], F32, kind="ExternalInput")
    tbd = nc.dram_tensor("tail_bias", [TAIL_P, 1], F32, kind="ExternalInput")
    v2d = nc.dram_tensor("v2pack", [PH, NPAIR * O], BF16, kind="ExternalInput")
    vtd = nc.dram_tensor("v2tail", [TAIL_P, O], BF16, kind="ExternalInput")
    c2d = nc.dram_tensor("c2sum", [O, 1], F32, kind="ExternalInput")
    outd = nc.dram_tensor("out", [O, NC], F32, kind="ExternalOutput")

    with tile.TileContext(nc) as tc, ExitStack() as ctx:
        const = ctx.enter_context(tc.tile_pool(name="const", bufs=1))
        hpool = ctx.enter_context(tc.tile_pool(name="h", bufs=4))
        hopool = ctx.enter_context(tc.tile_pool(name="ho", bufs=3))
        sbpool = ctx.enter_context(tc.tile_pool(name="sb", bufs=1))

        # Dummy 1-col tanh issued first: walrus places the ~2.7us
        # ACT_TABLE_LOAD before it, overlapping the load with input DMAs.
        dummy = const.tile([PH, 1], F32)
        nc.vector.memset(dummy[:], 0.0)
        nc.scalar.activation(out=dummy[:], in_=dummy[:], func=Tanh)

        # Inputs the first real activation needs, on separate DMA queues.
        w1c = const.tile([PH, OQ], F32)
        nc.gpsimd.dma_start(out=w1c[:], in_=w1d[:])
        b1c = const.tile([PH, OQ], F32)
        nc.gpsimd.dma_start(out=b1c[:], in_=b1d[:])
        xr = const.tile([PH, NC], F32)
        nc.sync.dma_start(out=xr[:], in_=xd[:])
        w2m = const.tile([PH, OQ * PH], BF16)
        nc.gpsimd.dma_start(out=w2m[:], in_=w2d[:])
        psc = const.tile([PH, NPAIR], F32)
        nc.gpsimd.dma_start(out=psc[:], in_=psd[:])
        pbc = const.tile([PH, NPAIR], F32)
        nc.gpsimd.dma_start(out=pbc[:], in_=pbd[:])
        tsc = const.tile([TAIL_P, 1], F32)
        nc.gpsimd.dma_start(out=tsc[:], in_=tsd[:])
        tbc = const.tile([TAIL_P, 1], F32)
        nc.gpsimd.dma_start(out=tbc[:], in_=tbd[:])
        v2p = const.tile([PH, NPAIR * O], BF16)
        nc.gpsimd.dma_start(out=v2p[:], in_=v2d[:])
        v2t = const.tile([TAIL_P, O], BF16)
        nc.gpsimd.dma_start(out=v2t[:], in_=vtd[:])
        c2s = const.tile([O, 1], F32)
        nc.gpsimd.dma_start(out=c2s[:], in_=c2d[:])

        # ---- layer 1: h = tanh(W1*x + b1); masked matmuls -> pre ----
        pre_sb = sbpool.tile([PH, NC], F32)
        with tc.tile_pool(name="pre", bufs=1, space="PSUM") as prepool:
            pre = prepool.tile([PH, NC], F32)
            for oq in range(OQ):
                h = hpool.tile([PH, NC], BF16, tag="h")
                nc.scalar.activation(
                    out=h[:],
                    in_=xr[:],
                    func=Tanh,
                    bias=b1c[:, oq : oq + 1],
                    scale=w1c[:, oq : oq + 1],
                )
                for j in range(NJ):
                    nc.tensor.matmul(
                        pre[:, j * MM_N : (j + 1) * MM_N],
                        w2m[:, oq * PH : (oq + 1) * PH],
                        h[:, j * MM_N : (j + 1) * MM_N],
                        start=(oq == 0),
                        stop=(oq == OQ - 1),
                    )
            # one copy frees all 8 PSUM banks; rows 68..127 already hold
            # the oq<60 duplicate thanks to the M=128 masked weights
            nc.vector.tensor_copy(out=pre_sb[:], in_=pre[:])

        # replicate rows 60..67 eight times for the leftover instruction
        # (DMA handles the partition remap; the DMA engines are idle here)
        pre_tail = sbpool.tile([TAIL_P, NC], F32)
        for r in range(NPAIR):
            nc.gpsimd.dma_start(
                out=pre_tail[r * TAIL_OQ : (r + 1) * TAIL_OQ, :],
                in_=pre_sb[DUP:OQ, :],
            )

        # ---- layer 2: packed tanh + masked matmuls -> out[o, n] ----
        outsb = sbpool.tile([O, NC], F32)
        with tc.tile_pool(name="ops", bufs=8, space="PSUM") as opspool:
            ops_tiles = [
                opspool.tile([O, MM_N], F32, tag="ops", name=f"ops_{j}")
                for j in range(NJ)
            ]
            for k in range(NPAIR):
                ho = hopool.tile([PH, NC], BF16, tag="ho")
                nc.scalar.activation(
                    out=ho[:],
                    in_=pre_sb[:],
                    func=Tanh,
                    bias=pbc[:, k : k + 1],
                    scale=psc[:, k : k + 1],
                )
                for j in range(NJ):
                    nc.tensor.matmul(
                        ops_tiles[j][:],
                        v2p[:, k * O : (k + 1) * O],
                        ho[:, j * MM_N : (j + 1) * MM_N],
                        start=(k == 0),
                        stop=False,
                    )
            hot = hopool.tile([TAIL_P, NC], BF16, tag="hot")
            nc.scalar.activation(
                out=hot[:],
                in_=pre_tail[:],
                func=Tanh,
                bias=tbc[:],
                scale=tsc[:],
            )
            for j in range(NJ):
                nc.tensor.matmul(
                    ops_tiles[j][:],
                    v2t[:],
                    hot[:, j * MM_N : (j + 1) * MM_N],
                    start=False,
                    stop=True,
                )
            for j in range(NJ):
                nc.vector.tensor_scalar_add(
                    out=outsb[:, j * MM_N : (j + 1) * MM_N],
                    in0=ops_tiles[j][:],
                    scalar1=c2s[:],
                )
        nc.sync.dma_start(out=outd[:], in_=outsb[:])

    nc.compile()
    return nc


def _prep_inputs(x, W1, b1, W2, b2, V1, c1, V2, c2):
    bf16 = ml_dtypes.bfloat16
    f32 = np.float32
    x = np.asarray(x, f32)
    # x_rep[c][(p*H+hh), n] = x[c*NC+n, p]
    xr = x.reshape(N_CORES, NC, P).transpose(0, 2, 1)  # (cores, P, NC)
    x_rep = np.ascontiguousarray(np.repeat(xr, H, axis=1), dtype=f32)

    w1col = np.ascontiguousarray(
        np.asarray(W1, f32).transpose(2, 3, 0, 1).reshape(PH, OQ)
    )
    b1col = np.ascontiguousarray(
        np.asarray(b1, f32).transpose(2, 3, 0, 1).reshape(PH, OQ)
    )
    # masked stationary weights, M=128: column j adds W2[oqj] to PSUM row j,
    # where oqj = j for j<68 and j-68 for j>=68 (duplicate rows for oq<60)
    w2t = np.asarray(W2, f32).transpose(2, 3, 0, 1).reshape(PH, OQ)  # [ph, oq]
    oq_of_row = np.concatenate([np.arange(OQ), np.arange(DUP)])  # (128,)
    w2mask = np.zeros((PH, OQ, PH), f32)
    for j in range(PH):
        w2mask[:, oq_of_row[j], j] = w2t[:, oq_of_row[j]]
    w2mask = np.ascontiguousarray(w2mask.reshape(PH, OQ * PH)).astype(bf16)

    b2sum = np.asarray(b2, f32).sum(axis=2).reshape(OQ)
    v1col = np.asarray(V1, f32).reshape(OQ, H)
    bias2 = np.asarray(c1, f32).reshape(OQ, H) + v1col * b2sum[:, None]

    # pair instruction k: partition j<68 -> (oq=j, hh=2k); j>=68 -> (oq=j-68, hh=2k+1)
    hh_of_row = np.where(np.arange(PH) < OQ, 0, 1)  # parity offset
    pair_scale = np.empty((PH, NPAIR), f32)
    pair_bias = np.empty((PH, NPAIR), f32)
    for k in range(NPAIR):
        hh = 2 * k + hh_of_row
        pair_scale[:, k] = v1col[oq_of_row, hh]
        pair_bias[:, k] = bias2[oq_of_row, hh]

    # leftover instruction: partition j2 -> (oq = 60 + j2%8, hh = 2*(j2//8)+1)
    j2 = np.arange(TAIL_P)
    t_oq = DUP + (j2 % TAIL_OQ)
    t_hh = 2 * (j2 // TAIL_OQ) + 1
    tail_scale = np.ascontiguousarray(v1col[t_oq, t_hh].reshape(TAIL_P, 1))
    tail_bias = np.ascontiguousarray(bias2[t_oq, t_hh].reshape(TAIL_P, 1))

    # layer-2 masked weights
    v2r = np.asarray(V2, f32).reshape(OQ, H)
    o_of_oq = np.repeat(np.arange(O), Q)
    v2pack = np.zeros((PH, NPAIR, O), f32)
    for k in range(NPAIR):
        hh = 2 * k + hh_of_row
        v2pack[np.arange(PH), k, o_of_oq[oq_of_row]] = v2r[oq_of_row, hh]
    v2pack = np.ascontiguousarray(v2pack.reshape(PH, NPAIR * O)).astype(bf16)
    v2tail = np.zeros((TAIL_P, O), f32)
    v2tail[j2, o_of_oq[t_oq]] = v2r[t_oq, t_hh]
    v2tail = np.ascontiguousarray(v2tail).astype(bf16)

    c2sum = np.ascontiguousarray(np.asarray(c2, f32).sum(axis=1).reshape(O, 1))

    shared = {
        "w1col": w1col,
        "b1col": b1col,
        "w2mask": w2mask,
        "pair_scale": pair_scale,
        "pair_bias": pair_bias,
        "tail_scale": tail_scale,
        "tail_bias": tail_bias,
        "v2pack": v2pack,
        "v2tail": v2tail,
        "c2sum": c2sum,
    }
    in_maps = [dict(shared, x_rep=np.ascontiguousarray(x_rep[c])) for c in range(N_CORES)]
    return in_maps


def run_spmd(x, W1, b1, W2, b2, V1, c1, V2, c2, trace=False):
    """Compile (cached), run on 8 cores, return (out_full, BassKernelResults)."""
    from concourse.bass_utils import run_bass_kernel_spmd

    if "nc" not in _CACHE:
        _CACHE["nc"] = _build()
    nc = _CACHE["nc"]
    in_maps = _prep_inputs(x, W1, b1, W2, b2, V1, c1, V2, c2)
    res = run_bass_kernel_spmd(nc, in_maps, list(range(N_CORES)), trace=trace)
    out_full = np.empty((N, O), dtype=np.float32)
    for c in range(N_CORES):
        out_full[c * NC : (c + 1) * NC, :] = res.results[c]["out"].T
    return out_full, res


def kernel(x, W1, b1, W2, b2, V1, c1, V2, c2):
    out, _ = run_spmd(x, W1, b1, W2, b2, V1, c1, V2, c2, trace=False)
    return out
